# revision 1
# baseline (speedup 1.0000x reference)
"""AssimilationLoss Trainium2 kernel.

Reference math (x: [B, N, D] f32):
    loss = mean_b || sum_i x[b,i,:] / max(||x[b,i,:]||, eps) ||^2 / N^2

Sharding: data-parallel over B across 8 NeuronCores (one batch element per
core).  Each core streams its [N, D] shard once from HBM (16 MiB -> memory
bound), computes partial_b = || sum_i x_i/||x_i|| ||^2 locally, and the host
averages the 8 scalars.

Per-core pipeline over [128, 512] row-tiles (raw Bacc, manual semaphores):
  DMA : hybrid plan -- first chunks via HWDGE (f32r storage, starts ~2.5us),
        bulk via SWDGE with f32->bf16 cast on the wire (halves SBUF writes);
        big chunks early, 1-tile chunks last for a short tail.
  ACT : activation(Square, accum_out)     -> ss[p] = sum_d x[p,d]^2   (3/8 of tiles)
  DVE : affine_mul_reduce (custom op)     -> ss[p]                    (5/8 of tiles)
  ACT : sqrt (batched over tile groups)   -> norm[p]
  DVE : reciprocal                        -> inv[p] = 1/||x_p||  (bf16 / f32r)
  PE  : matmul(lhsT=inv, rhs=x_tile)      -> s[1, D] += sum_p x[p,:]/||x_p||
        (the per-row scaling rides the matmul weights; PSUM accumulates)
Epilogue: ACT square+acc of s -> scalar, DMA out from the ACT HWDGE ring.
Measured on silicon: ~56-64 us/core vs a ~47 us HBM wire floor.
"""

import numpy as np

import concourse.bacc as bacc
import concourse.mybir as mybir
import concourse.tile as tile
from concourse.bass_utils import run_bass_kernel_spmd


def _ensure_ntff_hook():
    """Provide antenv.axon_hooks (NTFF profiling glue) if the image lacks it."""
    try:
        from antenv.axon_hooks import get_axon_ntff_profile_hook  # noqa: F401

        return
    except ImportError:
        pass
    import contextlib
    import ctypes
    import sys
    import types

    so_path = "/opt/axon/libaxon_pjrt.so"
    mod = types.ModuleType("antenv.axon_hooks")
    _state = {"hook": None}
    mod.set_axon_ntff_profile_hook = lambda h: _state.__setitem__("hook", h)
    mod.get_axon_ntff_profile_hook = lambda: _state["hook"]
    try:
        lib = ctypes.CDLL(so_path)
        if hasattr(lib, "axon_start_nrt_profile"):
            lib.axon_start_nrt_profile.argtypes = [
                ctypes.POINTER(ctypes.c_int64),
                ctypes.c_size_t,
            ]
            lib.axon_start_nrt_profile.restype = ctypes.c_int64
            lib.axon_stop_nrt_profile.argtypes = [ctypes.c_char_p]
            lib.axon_stop_nrt_profile.restype = ctypes.c_int64

            @contextlib.contextmanager
            def _hook(output_dir, device_ids):
                import jax

                jax.devices()
                if device_ids:
                    ids = (ctypes.c_int64 * len(device_ids))(*device_ids)
                    rc = lib.axon_start_nrt_profile(ids, len(device_ids))
                else:
                    rc = lib.axon_start_nrt_profile(None, 0)
                if rc != 0:
                    raise RuntimeError(f"axon_start_nrt_profile rc={rc}")
                try:
                    yield
                finally:
                    n = lib.axon_stop_nrt_profile(str(output_dir).encode())
                    if n <= 0:
                        print(f"ntff profile: rc={n} (no files?)", file=sys.stderr)

            _state["hook"] = _hook
    except OSError:
        pass
    import antenv

    sys.modules["antenv.axon_hooks"] = mod
    antenv.axon_hooks = mod


_ensure_ntff_hook()

B, N, D = 8, 8192, 512
P = 128                      # SBUF partitions
ROWS_PER_CHUNK = 512         # rows DMA'd per transfer (1 MiB read)
N_SUB = ROWS_PER_CHUNK // P  # row-tiles per chunk
N_CHUNKS = N // ROWS_PER_CHUNK

F32 = mybir.dt.float32
F32R = mybir.dt.float32r
BF16 = mybir.dt.bfloat16

# row-tiles (of N_SUB per chunk) whose square+rowsum runs on ACT; rest on DVE
ACT_SUBTILES = {3}


USE_RAW = True


def _build_nc():
    nc = bacc.Bacc("TRN2", target_bir_lowering=False, debug=False)
    x_dt = F32R if USE_RAW else F32
    x_ext = nc.dram_tensor("x", [N, D], x_dt, kind="ExternalInput")
    out_ext = nc.dram_tensor("out", [1, 1], F32, kind="ExternalOutput")

    if USE_RAW:
        _body_raw(nc, x_ext.ap(), out_ext.ap())
    else:
        with tile.TileContext(nc) as tc:
            _body(tc, nc, x_ext.ap(), out_ext.ap())

    nc.compile()
    return nc


def _act_set(c):
    """Subtile indices (within a chunk) whose square+rowsum runs on ACT."""
    return ACT_SUBTILES if c % 2 else ACT_SUBTILES | {1}


# DMA plan: entries (n_slots, kind, pc).  Each entry is one dma_start moving
# n_slots row-tiles onto partitions [0, pc).  kind: "hs" = HWDGE from sync
# (f32r storage, feeds the PE directly and starts before the SWDGE preamble
# finishes), "ha" = HWDGE from scalar, "sw" = SWDGE from gpsimd with an
# f32 -> bf16 cast on the wire (halves the SBUF write traffic).  Big chunks
# early, tiny chunks last so the compute tail after the final DMA is short.
DMA_PLAN = (
    [(3, "hs", 128), (3, "hs", 128)]
    + [(8, "sw", 128)] * 4
    + [(4, "sw", 128)] * 3
    + [(2, "sw", 128)] * 5
    + [(1, "sw", 128)] * 4
)

GROUP = 4  # tiles per sqrt/recip batch (never spans a kind change)

# dma_idx -> group index after whose squares the scalar engine issues that
# "ha" chunk (mid-stream issue paces aggregate HBM demand; an upfront issue
# would land the f32 chunks too early to smooth anything).
HA_ISSUE_AFTER_GROUP = {}


# global row-tile index -> engine for the square+rowsum (3 of 8 on ACT)
def _on_act(t):
    return t % 8 in (1, 4, 6)


def _body_raw(nc, x, out):
    """Raw Bacc version: manual semaphores, no Tile prologue/epilogue."""
    assert sum(m * pc for m, _, pc in DMA_PLAN) == N

    # per-DMA sbuf storage + tile map
    dmas = []  # (kind, ap, row0, m, pc)
    tiles = []  # (dma_idx, i_in_dma, ap, kind, pc)
    r0 = 0
    for di, (m, kind, pc) in enumerate(DMA_PLAN):
        dt = BF16 if kind == "sw" else F32R
        ap = nc.alloc_sbuf_tensor(f"xt{di}", [pc, m, D], dt).ap()
        dmas.append((kind, ap, r0, m, pc))
        for i in range(m):
            tiles.append((di, i, ap, kind, pc))
        r0 += m * pc
    assert r0 == N
    NT = len(tiles)

    # compute groups for sqrt/recip batching (same matmul-weight dtype only).
    # Uniform groups: the final group's first 3 amrs overlap the still-
    # streaming DMA, leaving one amr + one sqrt/recip/mm chain after the
    # last byte lands.  (1-tile tail groups measured WORSE: each pays the
    # full serial chain.)
    # ... except the very last tile, which gets its own group: its 1-tile
    # chain (amr -> sqrt -> recip -> mm) is all that remains after the final
    # byte lands, while the preceding group's chain completes in-stream.
    groups = []  # (tile0, gsize, kind)
    t = 0
    while t < NT:
        kind = tiles[t][3]
        if t == NT - 1:
            cap = 1
        elif t + GROUP > NT - 1:
            cap = NT - 1 - t
        else:
            cap = GROUP
        g = 1
        while g < cap and t + g < NT and tiles[t + g][3] == kind:
            g += 1
        groups.append((t, g, kind))
        t += g

    ss = nc.alloc_sbuf_tensor("ss", [P, NT], F32).ap()
    nrm = nc.alloc_sbuf_tensor("nrm", [P, NT], F32).ap()
    inv_r = nc.alloc_sbuf_tensor("inv_r", [P, NT], F32R).ap()
    inv_b = nc.alloc_sbuf_tensor("inv_b", [P, NT], BF16).ap()
    ss_b = nc.alloc_sbuf_tensor("ss_b", [P, 1], F32).ap()
    sq_a = nc.alloc_sbuf_tensor("sq_a", [P, D], F32).ap()
    sq_v = nc.alloc_sbuf_tensor("sq_v", [P, D], F32).ap()
    s_sq = nc.alloc_sbuf_tensor("s_sq", [1, D], F32).ap()
    partial = nc.alloc_sbuf_tensor("partial", [1, 1], F32).ap()

    import contextlib

    _stack = contextlib.ExitStack()
    with (
        _stack,
        nc.psum_tensor([1, D], F32) as s_acc,
        nc.semaphore("amr_sem") as amr_sem,
        nc.semaphore("ssq_sem") as ssq_sem,
        nc.semaphore("norm_sem") as norm_sem,
        nc.semaphore("inv_sem") as inv_sem,
        nc.semaphore("mm_sem") as mm_sem,
        nc.semaphore("fin_sem") as fin_sem,
        nc.semaphore("out_sem") as out_sem,
        nc.Block() as block,
    ):
        dma_sems = [
            _stack.enter_context(nc.semaphore(f"dma{i}"))
            for i in range(len(DMA_PLAN))
        ]

        def dma_src(di):
            kind, ap, r0, m, pc = dmas[di]
            return x[r0 : r0 + m * pc, :].rearrange("(p n) d -> p n d", p=pc)

        def issue(eng, want):
            for di, (kind, ap, r0, m, pc) in enumerate(dmas):
                if kind == want:
                    eng.dma_start(out=ap, in_=dma_src(di)).then_inc(
                        dma_sems[di], 16
                    )

        @block.sync
        def _(sync):
            issue(sync, "hs")
            # store from the warm sync HWDGE ring (a cold ring's first
            # trigger costs ~1.1us on its sequencer)
            sync.wait_ge(fin_sem, 1)
            sync.dma_start(out=out, in_=partial).then_inc(out_sem, 16)
            sync.wait_ge(out_sem, 16)

        @block.gpsimd
        def _(gpsimd):
            issue(gpsimd, "sw")

        @block.scalar
        def _(scalar):
            # Dummy activations: pull the ACT table loads (Square/Sqrt sets)
            # into the DMA flight time instead of the first real square.
            scalar.activation(
                out=sq_a[:1, :1],
                in_=s_sq[:1, :1],
                func=mybir.ActivationFunctionType.Square,
            )
            scalar.activation(
                out=sq_a[:1, :1],
                in_=s_sq[:1, :1],
                func=mybir.ActivationFunctionType.Sqrt,
            )
            for adi, (akind, aap, _r, _m, _p) in enumerate(dmas):
                if akind == "ha" and adi not in HA_ISSUE_AFTER_GROUP:
                    scalar.dma_start(out=aap, in_=dma_src(adi)).then_inc(
                        dma_sems[adi], 16
                    )

            def issue_pending_ha(gi):
                for adi, g in HA_ISSUE_AFTER_GROUP.items():
                    if g == gi:
                        aap = dmas[adi][1]
                        scalar.dma_start(out=aap, in_=dma_src(adi)).then_inc(
                            dma_sems[adi], 16
                        )

            last_dma_waited = [-1]

            def tile_wait(t):
                di = tiles[t][0]
                if di > last_dma_waited[0]:
                    scalar.wait_ge(dma_sems[di], 16)
                    last_dma_waited[0] = di

            def squares(gi):
                gt0, gsize, kind = groups[gi]
                for t in range(gt0, gt0 + gsize):
                    if t == NT - 1 and not _on_act(t):
                        # final tile: ACT squares the second free-dim half in
                        # parallel with DVE's first-half amr (shorter tail)
                        tile_wait(t)
                        di, i, ap, kind, pc = tiles[t]
                        apf = ap.bitcast(F32) if kind != "sw" else ap
                        scalar.activation(
                            out=sq_a[:pc, : D // 2],
                            in_=apf[:, i, D // 2 :],
                            func=mybir.ActivationFunctionType.Square,
                            accum_out=ss_b[:pc, :],
                        ).then_inc(ssq_sem, 1)
                    elif _on_act(t):
                        tile_wait(t)
                        di, i, ap, kind, pc = tiles[t]
                        apf = ap.bitcast(F32) if kind != "sw" else ap
                        scalar.activation(
                            out=sq_a[:pc, :],
                            in_=apf[:, i, :],
                            func=mybir.ActivationFunctionType.Square,
                            accum_out=ss[:pc, t : t + 1],
                        ).then_inc(ssq_sem, 1)

            def sqrt(gi):
                gt0, gsize, kind = groups[gi]
                scalar.wait_ge(amr_sem, gi + 1)
                scalar.activation(
                    out=nrm[:, gt0 : gt0 + gsize],
                    in_=ss[:, gt0 : gt0 + gsize],
                    func=mybir.ActivationFunctionType.Sqrt,
                ).then_inc(norm_sem, 1)

            squares(0)
            issue_pending_ha(0)
            for gi in range(1, len(groups)):
                squares(gi)
                issue_pending_ha(gi)
                sqrt(gi - 1)
            sqrt(len(groups) - 1)

            # epilogue: partial = sum_d s[d]^2, then DMA it out (HWDGE on the
            # sync ring; scalar only signals fin).
            scalar.wait_ge(mm_sem, len(groups))
            scalar.activation(
                out=s_sq,
                in_=s_acc.ap(),
                func=mybir.ActivationFunctionType.Square,
                accum_out=partial,
            ).then_inc(fin_sem, 1)

        @block.vector
        def _(vector):
            n_act = 0
            last_dma_waited = [-1]

            def tile_wait(t):
                di = tiles[t][0]
                if di > last_dma_waited[0]:
                    vector.wait_ge(dma_sems[di], 16)
                    last_dma_waited[0] = di

            def amrs(gi):
                nonlocal n_act
                gt0, gsize, kind = groups[gi]
                need_ssq_wait = False
                for t in range(gt0, gt0 + gsize):
                    if t == NT - 1 and not _on_act(t):
                        tile_wait(t)
                        di, i, ap, kind, pc = tiles[t]
                        apf = ap.bitcast(F32) if kind != "sw" else ap
                        vector.affine_mul_reduce(
                            out=sq_v[:pc, : D // 2],
                            accum_out=ss[:pc, t : t + 1],
                            in0=apf[:, i, : D // 2],
                            in1=apf[:, i, : D // 2],
                            scale=1.0,
                            bias=0.0,
                        )
                        n_act += 1  # ACT's half-square of this tile
                        vector.wait_ge(ssq_sem, n_act)
                        vector.tensor_add(
                            ss[:pc, t : t + 1], ss[:pc, t : t + 1], ss_b[:pc, :]
                        )
                        continue
                    if _on_act(t):
                        n_act += 1
                        need_ssq_wait = True
                        continue
                    tile_wait(t)
                    di, i, ap, kind, pc = tiles[t]
                    apf = ap.bitcast(F32) if kind != "sw" else ap
                    vector.affine_mul_reduce(
                        out=sq_v[:pc, :],
                        accum_out=ss[:pc, t : t + 1],
                        in0=apf[:, i, :],
                        in1=apf[:, i, :],
                        scale=1.0,
                        bias=0.0,
                    )
                if need_ssq_wait:
                    vector.wait_ge(ssq_sem, n_act)
                tile_wait(gt0 + gsize - 1)
                vector.engine_nop().then_inc(amr_sem, 1)

            def recip(gi):
                gt0, gsize, kind = groups[gi]
                inv = inv_b if kind == "sw" else inv_r
                vector.wait_ge(norm_sem, gi + 1)
                with nc.allow_low_precision(reason="matmul weight dtype"):
                    vector.reciprocal(
                        out=inv[:, gt0 : gt0 + gsize],
                        in_=nrm[:, gt0 : gt0 + gsize],
                    ).then_inc(inv_sem, 1)

            amrs(0)
            for gi in range(1, len(groups)):
                amrs(gi)
                recip(gi - 1)
            recip(len(groups) - 1)

        @block.tensor
        def _(tensor):
            mm = 0
            for gi, (gt0, gsize, kind) in enumerate(groups):
                inv = inv_b if kind == "sw" else inv_r
                tensor.wait_ge(inv_sem, gi + 1)
                for t in range(gt0, gt0 + gsize):
                    di, i, ap, kind2, pc = tiles[t]
                    instr = tensor.matmul(
                        s_acc.ap(),
                        inv[:pc, t : t + 1],
                        ap[:, i, :],
                        start=(mm == 0),
                        stop=(mm == NT - 1),
                    )
                    mm += 1
                    if t == gt0 + gsize - 1:
                        instr.then_inc(mm_sem, 1)


def _body(tc, nc, x, out):
    """TileContext version (reference/baseline)."""
    import contextlib

    ctx = contextlib.ExitStack()
    with ctx:
        data = ctx.enter_context(tc.tile_pool(name="data", bufs=N_CHUNKS))
        small = ctx.enter_context(tc.tile_pool(name="small", bufs=4))
        sq = ctx.enter_context(tc.tile_pool(name="sq", bufs=2))
        sqd = ctx.enter_context(tc.tile_pool(name="sqd", bufs=2))
        singles = ctx.enter_context(tc.tile_pool(name="singles", bufs=1))
        psum = ctx.enter_context(tc.tile_pool(name="psum", bufs=1, space="PSUM"))

        s_acc = psum.tile([1, D], F32)
        x_chunks = x.rearrange("(c p n) d -> c p n d", p=P, n=N_SUB)

        mm = 0
        for c in range(N_CHUNKS):
            xt = data.tile([P, N_SUB, D], BF16)
            nc.gpsimd.dma_start(out=xt, in_=x_chunks[c])

            act_subtiles = _act_set(c)
            ss = small.tile([P, N_SUB], F32, tag="ss")
            for n in range(N_SUB):
                if n in act_subtiles:
                    sq_t = sq.tile([P, D], BF16)
                    nc.scalar.activation(
                        out=sq_t,
                        in_=xt[:, n, :],
                        func=mybir.ActivationFunctionType.Square,
                        accum_out=ss[:, n : n + 1],
                    )
                else:
                    sq_d = sqd.tile([P, D], BF16)
                    nc.vector.affine_mul_reduce(
                        out=sq_d,
                        accum_out=ss[:, n : n + 1],
                        in0=xt[:, n, :],
                        in1=xt[:, n, :],
                        scale=1.0,
                        bias=0.0,
                    )

            rcp = small.tile([P, N_SUB], F32, tag="rcp")
            nc.vector.reciprocal(out=rcp, in_=ss)
            inv = small.tile([P, N_SUB], BF16, tag="inv")
            nc.scalar.activation(
                out=inv, in_=rcp, func=mybir.ActivationFunctionType.Sqrt
            )

            for n in range(N_SUB):
                nc.tensor.matmul(
                    s_acc,
                    inv[:, n : n + 1],
                    xt[:, n, :],
                    start=(mm == 0),
                    stop=(mm == N_CHUNKS * N_SUB - 1),
                )
                mm += 1

        s_sq = singles.tile([1, D], F32)
        partial = singles.tile([1, 1], F32)
        nc.scalar.activation(
            out=s_sq,
            in_=s_acc,
            func=mybir.ActivationFunctionType.Square,
            accum_out=partial,
        )
        nc.sync.dma_start(out=out, in_=partial)


_NC_CACHE = {}


def _get_nc():
    if "nc" not in _NC_CACHE:
        _NC_CACHE["nc"] = _build_nc()
    return _NC_CACHE["nc"]


def kernel(x: np.ndarray, trace: bool = False):
    assert x.shape == (B, N, D), x.shape
    nc = _get_nc()
    in_maps = [{"x": np.ascontiguousarray(x[b], dtype=np.float32)} for b in range(B)]
    res = None
    for attempt in range(3):
        try:
            res = run_bass_kernel_spmd(
                nc, in_maps, core_ids=list(range(B)), trace=trace
            )
            break
        except Exception:
            # A previously crashed process can leave the accelerator in an
            # "unrecoverable" state for ~30s; it heals on its own.
            if attempt == 2:
                raise
            import time

            time.sleep(25)
    partials = [float(r["out"][0, 0]) for r in res.results]
    val = np.float32(np.sum(np.asarray(partials, dtype=np.float64)) / (N * N) / B)
    if trace:
        return val, res
    return val

